# revision 2
# baseline (speedup 1.0000x reference)
"""Distillation-loss kernel v3 for Trainium2 (Bass/Tile), 8 NeuronCores.

Per token t (vocab V=10000):
  lse = log(sum_v exp(x));  dot = sum_v x*soft;  sumlog = sum_v x;  ly = x[y]
  soft_tok = dot - lse;  hard_tok = c_y*ly + c_s*sumlog - lse

Sharding: valid tokens (t < ylen) packed, split evenly over 8 cores
(292-293 tokens/core -> 304 slots). Per-core layout is MIXED:
  - mini-block first: 48 slots in 8-way split (token spans 8 partitions x
    1250 cols; 3 column-groups of 16 tokens) -> cols [0, 3750)
  - 2 big tiles: tokens on partitions, [128, 10000] each -> cols [3750, 23750)
    (all 256 big tokens are valid; only the mini-block has pads, which are
    zero and self-mask everywhere except lse, masked by w16)

Engines (fp8 transfers, measured rates):
  ACT : exp. Mini -> E bf16 (no accum); big chunks -> fp8 junk + f32 accum
        (per-token sums since tokens own partitions).  ~22us = bottleneck
  DVE : dot = stt(X,S) per chunk w/ accum col; mini segmented sumexp reduce
  PE  : sumlog = ones^T X via fp8 DoubleRow matmuls; tiny epilogue matmuls
  Host: s_y = sum w*x[y] (O(nv) gather); fp8 casts; final combine
"""

import math
from contextlib import ExitStack

import numpy as np
import ml_dtypes

import concourse.bacc as bacc
import concourse.tile as tile
from concourse import mybir
from concourse.bass_utils import run_bass_kernel_spmd

VOCAB = 10000
SOFT_W = 0.5
LSM = 0.1
NCORES = 8

P = 128
SPLIT = 8                  # mini-block: partitions per token
TPG = P // SPLIT           # tokens per mini column-group = 16
C = VOCAB // SPLIT         # mini group width = 1250
MG = 3                     # mini groups
MINIW = MG * C             # 3750
NBIG = 2                   # big tiles
W = MINIW + NBIG * VOCAB   # 23750
SLOTS = NBIG * P + MG * TPG  # 304
SSCALE = 8192.0            # soft-label scale (pow2: lossless to undo)

F32 = mybir.dt.float32
BF16 = mybir.dt.bfloat16
FP8 = mybir.dt.float8e4

NP_F8 = np.dtype(ml_dtypes.float8_e4m3)

_PROG_CACHE: dict = {}
LAST_RESULT = None


def _act_tables_ln_exp(arch):
    """Restrict to the activation-table set holding BOTH Exp and Ln so the
    kernel pays a single ACT_TABLE_LOAD."""
    import concourse.hw_specs as hw_specs

    full = hw_specs.get_activation_tables(arch)
    return {
        name: (funcs if name == "natural_log_exp_and_others" else set())
        for name, funcs in full.items()
    }


def _chunks():
    """DMA/compute chunks: mini first, then big tiles split in 5000-col halves."""
    out = [(0, MINIW)]
    for t in range(NBIG):
        base = MINIW + t * VOCAB
        out.append((base, VOCAB // 2))
        out.append((base + VOCAB // 2, VOCAB // 2))
    return out


def _build():
    nc = bacc.Bacc("TRN2", target_bir_lowering=False, debug=False)

    xd = nc.dram_tensor("xd", [P, W], FP8, kind="ExternalInput").ap()
    sd = nc.dram_tensor("sd", [P, W], FP8, kind="ExternalInput").ap()
    # [128, 16] octet-indicator (f32) for the mini partition-reduce
    p8 = nc.dram_tensor("p8", [P, TPG], F32, kind="ExternalInput").ap()
    # [16, MG] f32 valid mask for mini tokens
    wd = nc.dram_tensor("wd", [TPG, MG], F32, kind="ExternalInput").ap()
    out = nc.dram_tensor("out", [1, 4], F32, kind="ExternalOutput").ap()

    AF = mybir.ActivationFunctionType
    OP = mybir.AluOpType
    AX = mybir.AxisListType

    chunks = _chunks()
    nch = len(chunks)          # 5
    nbch = nch - 1             # big chunks

    with tile.TileContext(nc) as tc, ExitStack() as ctx:
        pool = ctx.enter_context(tc.tile_pool(name="pool", bufs=1))
        psum = ctx.enter_context(tc.tile_pool(name="psum", bufs=1, space="PSUM"))

        X = pool.tile([P, W], FP8, tag="X")
        S = pool.tile([P, W], FP8, tag="S")
        Em = pool.tile([P, MINIW], BF16, tag="Em")      # mini exp
        JA = pool.tile([P, VOCAB // 2], FP8, tag="JA")  # ACT junk (big chunks)
        JV = pool.tile([P, VOCAB // 2], FP8, tag="JV")  # DVE junk
        JVm = pool.tile([P, MINIW], FP8, tag="JVm")     # DVE junk (mini)
        sacc = pool.tile([P, nbch], F32, tag="sacc")    # big exp accums
        dacc = pool.tile([P, nch], F32, tag="dacc")     # dot accums
        sem = pool.tile([P, MG], F32, tag="sem")        # mini group sums
        se_big = pool.tile([P, NBIG], F32, tag="se_big")
        lse_big = pool.tile([P, NBIG], F32, tag="lse_big")
        red2 = pool.tile([P, 2], F32, tag="red2")       # col0: dot rowsum, col1: lse rowsum
        p8t = pool.tile([P, TPG], F32, tag="p8t")
        wt = pool.tile([TPG, MG], F32, tag="wt")
        lse16 = pool.tile([TPG, MG], F32, tag="lse16")
        wl16 = pool.tile([TPG, 1], F32, tag="wl16")
        ones16 = pool.tile([TPG, 1], F32, tag="ones16")
        onesP = pool.tile([P, 1], F32, tag="onesP")
        ones8 = pool.tile([P, 32], FP8, tag="ones8")
        ot = pool.tile([1, 4], F32, tag="ot")

        pm = psum.tile([1, 512], F32, tag="pm")          # sumlog accum
        p2 = psum.tile([1, 2], F32, tag="p2")            # (dot, lse_big_sum)
        p16 = psum.tile([TPG, MG], F32, tag="p16")       # mini octet sums
        pw = psum.tile([1, 1], F32, tag="pw")            # mini wlse

        nc.scalar.dma_start(p8t[:], p8[:])
        nc.scalar.dma_start(wt[:], wd[:])
        nc.vector.memset(ones8[:], 1.0)
        nc.vector.memset(ones16[:], 1.0)
        nc.vector.memset(onesP[:], 1.0)

        # ---- streaming DMA + compute ----
        for ci, (c0, cw) in enumerate(chunks):
            cs = slice(c0, c0 + cw)
            nc.sync.dma_start(X[:, cs], xd[:, cs])
            nc.gpsimd.dma_start(S[:, cs], sd[:, cs])
            if ci == 0:
                nc.scalar.activation(Em[:, :], X[:, cs], AF.Exp)
            else:
                nc.scalar.activation(
                    JA[:, :cw], X[:, cs], AF.Exp,
                    accum_out=sacc[:, ci - 1 : ci],
                )
            jv = JVm if ci == 0 else JV
            nc.vector.scalar_tensor_tensor(
                jv[:, :cw], X[:, cs], 1.0, S[:, cs], OP.mult, OP.mult,
                accum_out=dacc[:, ci : ci + 1],
            )
            if ci == 0:
                nc.vector.tensor_reduce(
                    sem[:, :], Em[:].rearrange("p (g c) -> p g c", g=MG),
                    AX.X, OP.add,
                )

        # ---- PE sumlog sweep (fp8 DoubleRow; pads are zero so no mask) ----
        onesdr = ones8[:].rearrange("p (j m) -> p j m", j=2)[:, :, 0:1]
        mm = []
        pos = 0
        while pos + 1024 <= W:
            mm.append((pos, 1024, True))
            pos += 1024
        rem = W - pos                       # 198
        dr_rem = (rem // 32) * 32           # 192
        if dr_rem >= 32:
            mm.append((pos, dr_rem, True))
            pos += dr_rem
        if W - pos:
            mm.append((pos, W - pos, False))
        for i, (m0, mw, dr) in enumerate(mm):
            st = i == 0
            sp = i == len(mm) - 1
            if dr:
                rhs3 = X[:, m0 : m0 + mw].rearrange("p (j n) -> p j n", j=2)
                nc.tensor.matmul(pm[0:1, : mw // 2], onesdr, rhs3,
                                 start=st, stop=sp, perf_mode=mybir.MatmulPerfMode.DoubleRow)
            else:
                nc.tensor.matmul(pm[0:1, :mw], ones8[:, 0:1], X[:, m0 : m0 + mw],
                                 start=st, stop=sp)

        # ---- mini epilogue: octet sums, Ln, mask, reduce ----
        # (after the PE sweep so the in-order PE/ACT queues don't stall on it)
        nc.tensor.matmul(p16[:, :], p8t[:, :], sem[:, :], start=True, stop=True)
        nc.scalar.activation(lse16[:, :], p16[:, :], AF.Ln)
        nc.vector.scalar_tensor_tensor(
            lse16[:, :], lse16[:, :], 1.0, wt[:, :], OP.mult, OP.mult,
            accum_out=wl16[:, 0:1],
        )
        nc.tensor.matmul(pw[0:1, :], ones16[:, 0:1], wl16[:, 0:1],
                         start=True, stop=True)

        # ---- epilogue (big tiles + totals) ----
        nc.vector.tensor_reduce(
            se_big[:, :], sacc[:].rearrange("p (t c) -> p t c", t=NBIG), AX.X, OP.add
        )
        nc.scalar.activation(lse_big[:, :], se_big[:, :], AF.Ln)
        nc.vector.tensor_reduce(red2[:, 0:1], dacc[:, :], AX.X, OP.add)
        nc.vector.tensor_reduce(red2[:, 1:2], lse_big[:, :], AX.X, OP.add)
        nc.tensor.matmul(p2[0:1, :], onesP[:, 0:1], red2[:, :], start=True, stop=True)

        nc.vector.tensor_copy(ot[0:1, 0:1], p2[0:1, 0:1])      # dot total
        nc.vector.tensor_reduce(ot[0:1, 1:2], pm[0:1, :], AX.X, OP.add)  # sumlog
        nc.vector.tensor_copy(ot[0:1, 2:3], p2[0:1, 1:2])      # big lse sum (w=1)
        nc.vector.tensor_copy(ot[0:1, 3:4], pw[0:1, :])        # mini w*lse sum
        nc.sync.dma_start(out[0:1, :], ot[0:1, :])

    orig_tables = bacc.get_activation_tables
    bacc.get_activation_tables = _act_tables_ln_exp
    try:
        nc.compile()
    finally:
        bacc.get_activation_tables = orig_tables
    return nc


def _get_prog():
    if "p" not in _PROG_CACHE:
        _PROG_CACHE["p"] = _build()
    return _PROG_CACHE["p"]


def _pack_core(tok_x, tok_s, n_valid):
    """tok_x/tok_s: [SLOTS, V] f32 (zero-padded).

    Slot order: slots 0..255 -> big tokens (tile t partition p = slot 128t+p);
    slots 256..303 -> mini token m: group m//16, octet m%16.
    Returns xr, sr [128, W] fp8 and w16 [16, MG] mask."""
    big = tok_x[: NBIG * P].reshape(NBIG, P, VOCAB)
    bigs = tok_s[: NBIG * P].reshape(NBIG, P, VOCAB)
    mini = tok_x[NBIG * P :].reshape(MG, TPG, SPLIT, C)
    minis = tok_s[NBIG * P :].reshape(MG, TPG, SPLIT, C)

    xr = np.empty((P, W), np.float32)
    sr = np.empty((P, W), np.float32)
    xr[:, :MINIW] = mini.transpose(1, 2, 0, 3).reshape(P, MINIW)
    sr[:, :MINIW] = minis.transpose(1, 2, 0, 3).reshape(P, MINIW)
    for t in range(NBIG):
        xr[:, MINIW + t * VOCAB : MINIW + (t + 1) * VOCAB] = big[t]
        sr[:, MINIW + t * VOCAB : MINIW + (t + 1) * VOCAB] = bigs[t]

    w16 = np.zeros((TPG, MG), np.float32)
    mini_idx = NBIG * P + np.arange(MG * TPG).reshape(MG, TPG).T
    w16[mini_idx < n_valid] = 1.0
    return xr.astype(NP_F8), sr.astype(NP_F8), w16


def _shard(logits, ys, soft_labels, ylens):
    B, T, V = logits.shape
    fl = np.asarray(logits).reshape(B * T, V)
    fs = np.asarray(soft_labels).reshape(B * T, V)
    fy = np.asarray(ys).reshape(B * T)
    yl = np.asarray(ylens).reshape(B)
    valid = (np.arange(T)[None, :] < yl[:, None]).reshape(B * T)
    idx = np.flatnonzero(valid)
    nv = int(idx.size)
    per = math.ceil(nv / NCORES)
    assert per <= SLOTS, f"tokens per core {per} exceed {SLOTS} slots"
    assert per > NBIG * P, "big tiles must be fully valid"

    # exact host-side s_y over valid tokens
    s_y = float(np.sum(fl[idx, fy[idx]], dtype=np.float64))

    p8 = np.zeros((P, TPG), np.float32)
    p8[np.arange(P), np.arange(P) // SPLIT] = 1.0

    in_maps = []
    for c in range(NCORES):
        sel = idx[c * per : (c + 1) * per]
        n = len(sel)
        tx = np.zeros((SLOTS, V), np.float32)
        ts = np.zeros((SLOTS, V), np.float32)
        tx[:n] = fl[sel]
        ts[:n] = fs[sel] * SSCALE
        xr, sr, w16 = _pack_core(tx, ts, n)
        in_maps.append({"xd": xr, "sd": sr, "p8": p8, "wd": w16})
    return in_maps, (B, V, s_y)


def _combine(per_core_outs, B, V, s_y):
    S = np.zeros(4, np.float64)
    for o in per_core_outs:
        S += np.asarray(o, dtype=np.float64).reshape(-1)
    s_dot = S[0] / SSCALE
    s_sumlog = S[1]
    s_wlse = S[2] + S[3]
    c_s = LSM / (V - 1)
    c_y = (1.0 - LSM) - c_s
    t_soft = s_dot - s_wlse
    t_hard = c_y * s_y + c_s * s_sumlog - s_wlse
    loss_soft = -t_soft / B
    loss_hard = -t_hard / B
    loss = SOFT_W * loss_soft + (1.0 - SOFT_W) * loss_hard
    return np.array([loss, loss_soft, loss_hard], dtype=np.float32)


def kernel(logits, ys, soft_labels, ylens):
    global LAST_RESULT
    logits = np.ascontiguousarray(np.asarray(logits), dtype=np.float32)
    soft_labels = np.ascontiguousarray(np.asarray(soft_labels), dtype=np.float32)
    in_maps, (B, V, s_y) = _shard(logits, ys, soft_labels, ylens)
    nc = _get_prog()
    res = run_bass_kernel_spmd(nc, in_maps, list(range(NCORES)))
    LAST_RESULT = res
    return _combine([r["out"] for r in res.results], B, V, s_y)


# ---------------- numpy simulation of the device program ----------------

def _simulate_core(xr, sr, w16):
    x = xr.astype(np.float64)
    s = sr.astype(np.float64)
    dot = (x * s).sum()
    sumlog = x.sum()
    # mini
    em = np.exp(x[:, :MINIW]).astype(ml_dtypes.bfloat16).astype(np.float64)
    sem = em.reshape(P, MG, C).sum(axis=2)
    se16 = sem.reshape(TPG, SPLIT, MG).sum(axis=1)
    wlse_mini = (np.log(se16) * w16).sum()
    # big
    lse_big = 0.0
    for t in range(NBIG):
        eb = np.exp(x[:, MINIW + t * VOCAB : MINIW + (t + 1) * VOCAB])
        lse_big += np.log(eb.sum(axis=1)).sum()
    return dot, sumlog, lse_big, wlse_mini


def simulate(logits, ys, soft_labels, ylens):
    in_maps, (B, V, s_y) = _shard(logits, ys, soft_labels, ylens)
    outs = [np.array(_simulate_core(m["xd"], m["sd"], m["wd"])) for m in in_maps]
    return _combine(outs, B, V, s_y)


if __name__ == "__main__":
    import reference

    ins = reference.setup_inputs()
    exp = np.asarray([float(v) for v in reference.reference(**ins)])
    got = simulate(**{k: np.asarray(v) for k, v in ins.items()})
    rel = np.max(np.abs(got.astype(np.float64) - exp) / np.abs(exp))
    print("expected:", exp)
    print("simulated:", got)
    print(f"sim relative error: {rel:.3e}")


# revision 3
# speedup vs baseline: 1.0454x; 1.0454x over previous
"""Distillation-loss kernel v3 for Trainium2 (Bass/Tile), 8 NeuronCores.

Per token t (vocab V=10000):
  lse = log(sum_v exp(x));  dot = sum_v x*soft;  sumlog = sum_v x;  ly = x[y]
  soft_tok = dot - lse;  hard_tok = c_y*ly + c_s*sumlog - lse

Sharding: valid tokens (t < ylen) packed, split evenly over 8 cores
(292-293 tokens/core -> 304 slots). Per-core layout is MIXED:
  - mini-block first: 48 slots in 8-way split (token spans 8 partitions x
    1250 cols; 3 column-groups of 16 tokens) -> cols [0, 3750)
  - 2 big tiles: tokens on partitions, [128, 10000] each -> cols [3750, 23750)
    (all 256 big tokens are valid; only the mini-block has pads, which are
    zero and self-mask everywhere except lse, masked by w16)

Engines (fp8 transfers, measured rates):
  ACT : exp. Mini -> E bf16 (no accum); big chunks -> fp8 junk + f32 accum
        (per-token sums since tokens own partitions).  ~22us = bottleneck
  DVE : dot = stt(X,S) per chunk w/ accum col; mini segmented sumexp reduce
  PE  : sumlog = ones^T X via fp8 DoubleRow matmuls; tiny epilogue matmuls
  Host: s_y = sum w*x[y] (O(nv) gather); fp8 casts; final combine
"""

import math
from contextlib import ExitStack

import numpy as np
import ml_dtypes

import concourse.bacc as bacc
import concourse.tile as tile
from concourse import mybir
from concourse.bass_utils import run_bass_kernel_spmd

VOCAB = 10000
SOFT_W = 0.5
LSM = 0.1
NCORES = 8

P = 128
SPLIT = 8                  # mini-block: partitions per token
TPG = P // SPLIT           # tokens per mini column-group = 16
C = VOCAB // SPLIT         # mini group width = 1250
MG = 3                     # mini groups
MINIW = MG * C             # 3750
NBIG = 2                   # big tiles
W = MINIW + NBIG * VOCAB   # 23750
SLOTS = NBIG * P + MG * TPG  # 304
SSCALE = 8192.0            # soft-label scale (pow2: lossless to undo)

F32 = mybir.dt.float32
BF16 = mybir.dt.bfloat16
FP8 = mybir.dt.float8e4

NP_F8 = np.dtype(ml_dtypes.float8_e4m3)

_PROG_CACHE: dict = {}
LAST_RESULT = None


def _act_tables_ln_exp(arch):
    """Restrict to the activation-table set holding BOTH Exp and Ln so the
    kernel pays a single ACT_TABLE_LOAD."""
    import concourse.hw_specs as hw_specs

    full = hw_specs.get_activation_tables(arch)
    return {
        name: (funcs if name == "natural_log_exp_and_others" else set())
        for name, funcs in full.items()
    }


# X DMA pieces: mini split in 3 (earlier ACT start), then 5000-col halves
X_CHUNKS = [(0, C), (C, C), (2 * C, C)] + [
    (MINIW + i * 5000, 5000) for i in range(2 * NBIG)
]
# S DMA pieces = dot-stt chunks (DVE is queue-bound at the tail, so no split)
S_CHUNKS = [(0, MINIW)] + [(MINIW + i * 5000, 5000) for i in range(2 * NBIG)]


def _build():
    nc = bacc.Bacc("TRN2", target_bir_lowering=False, debug=False)

    xd = nc.dram_tensor("xd", [P, W], FP8, kind="ExternalInput").ap()
    sd = nc.dram_tensor("sd", [P, W], FP8, kind="ExternalInput").ap()
    # [128, 16] octet-indicator (f32) for the mini partition-reduce
    p8 = nc.dram_tensor("p8", [P, TPG], F32, kind="ExternalInput").ap()
    # [16, MG] f32 valid mask for mini tokens
    wd = nc.dram_tensor("wd", [TPG, MG], F32, kind="ExternalInput").ap()
    out = nc.dram_tensor("out", [1, 4], F32, kind="ExternalOutput").ap()

    AF = mybir.ActivationFunctionType
    OP = mybir.AluOpType
    AX = mybir.AxisListType

    nbch = 2 * NBIG            # big 5000-col chunks

    with tile.TileContext(nc) as tc, ExitStack() as ctx:
        pool = ctx.enter_context(tc.tile_pool(name="pool", bufs=1))
        psum = ctx.enter_context(tc.tile_pool(name="psum", bufs=1, space="PSUM"))

        X = pool.tile([P, W], FP8, tag="X")
        S = pool.tile([P, W], FP8, tag="S")
        JA = pool.tile([P, VOCAB // 2], FP8, tag="JA")  # ACT junk
        JV = pool.tile([P, VOCAB // 2], FP8, tag="JV")  # DVE junk
        JVm = pool.tile([P, MINIW], FP8, tag="JVm")     # DVE junk (mini)
        sacc = pool.tile([P, nbch], F32, tag="sacc")    # big exp accums
        se_big = pool.tile([P, NBIG], F32, tag="se_big")
        dacc = pool.tile([P, nbch + 1], F32, tag="dacc")  # dot accums
        sem = pool.tile([P, MG], F32, tag="sem")        # mini group sums
        lse_big = pool.tile([P, NBIG], F32, tag="lse_big")
        red2 = pool.tile([P, 2], F32, tag="red2")       # col0: dot rowsum, col1: lse rowsum
        p8t = pool.tile([P, TPG], F32, tag="p8t")
        wt = pool.tile([TPG, MG], F32, tag="wt")
        lse16 = pool.tile([TPG, MG], F32, tag="lse16")
        wl16 = pool.tile([TPG, 1], F32, tag="wl16")
        ones16 = pool.tile([TPG, 1], F32, tag="ones16")
        onesP = pool.tile([P, 1], F32, tag="onesP")
        ones8 = pool.tile([P, 32], FP8, tag="ones8")
        jsum = pool.tile([1, 512], F32, tag="jsum")
        ot = pool.tile([1, 4], F32, tag="ot")

        pm = psum.tile([1, 512], F32, tag="pm")          # sumlog accum
        p2 = psum.tile([1, 2], F32, tag="p2")            # (dot, lse_big_sum)
        p16 = psum.tile([TPG, MG], F32, tag="p16")       # mini octet sums
        pw = psum.tile([1, 1], F32, tag="pw")            # mini wlse

        nc.scalar.dma_start(p8t[:], p8[:])
        nc.scalar.dma_start(wt[:], wd[:])
        nc.vector.memset(ones8[:], 1.0)
        nc.vector.memset(ones16[:], 1.0)
        nc.vector.memset(onesP[:], 1.0)

        # ---- DMA issue: X on sync queue, S on gpsimd queue, alternating ----
        def dma_x(i):
            c0, cw = X_CHUNKS[i]
            nc.sync.dma_start(X[:, c0 : c0 + cw], xd[:, c0 : c0 + cw])

        def dma_s(i):
            c0, cw = S_CHUNKS[i]
            nc.gpsimd.dma_start(S[:, c0 : c0 + cw], sd[:, c0 : c0 + cw])

        dma_x(0); dma_x(1); dma_x(2); dma_s(0)
        for i in range(nbch):
            dma_x(3 + i); dma_s(1 + i)

        # ---- ACT: mini exp as 3 per-group accum instrs (sem comes straight
        # from the f32 accumulator; no DVE reduce, and ACT starts ~1.5us
        # earlier on the first 1250-col piece) ----
        for g in range(MG):
            nc.scalar.activation(
                JA[:, 0:C], X[:, g * C : (g + 1) * C], AF.Exp,
                accum_out=sem[:, g : g + 1],
            )
        # mini octet sums on PE (dep-ready early; ahead of the sweep)
        nc.tensor.matmul(p16[:, :], p8t[:, :], sem[:, :], start=True, stop=True)

        # ---- big exps (Ln-mini after the 2nd so p16 has ample slack) ----
        for ci in range(nbch):
            cs = slice(MINIW + ci * 5000, MINIW + (ci + 1) * 5000)
            nc.scalar.activation(
                JA[:, :], X[:, cs], AF.Exp, accum_out=sacc[:, ci : ci + 1]
            )
            if ci == 1:
                nc.scalar.activation(lse16[:, :], p16[:, :], AF.Ln)

        # ---- DVE: dot stt chunks; wl16 slotted right after the mini stt so
        # the mini epilogue leaves the kernel tail entirely ----
        nc.vector.scalar_tensor_tensor(
            JVm[:, :], X[:, 0:MINIW], 1.0, S[:, 0:MINIW], OP.mult, OP.mult,
            accum_out=dacc[:, 0:1],
        )
        nc.vector.scalar_tensor_tensor(
            lse16[:, :], lse16[:, :], 1.0, wt[:, :], OP.mult, OP.mult,
            accum_out=wl16[:, 0:1],
        )
        nc.tensor.matmul(pw[0:1, :], ones16[:, 0:1], wl16[:, 0:1],
                         start=True, stop=True)
        for ci in range(nbch):
            cs = slice(MINIW + ci * 5000, MINIW + (ci + 1) * 5000)
            nc.vector.scalar_tensor_tensor(
                JV[:, :], X[:, cs], 1.0, S[:, cs], OP.mult, OP.mult,
                accum_out=dacc[:, ci + 1 : ci + 2],
            )

        # ---- PE sumlog sweep (fp8 DoubleRow; pads are zero so no mask) ----
        onesdr = ones8[:].rearrange("p (j m) -> p j m", j=2)[:, :, 0:1]
        mm = []
        pos = 0
        while pos + 1024 <= W:
            mm.append((pos, 1024, True))
            pos += 1024
        rem = W - pos                       # 198
        dr_rem = (rem // 32) * 32           # 192
        if dr_rem >= 32:
            mm.append((pos, dr_rem, True))
            pos += dr_rem
        if W - pos:
            mm.append((pos, W - pos, False))
        for i, (m0, mw, dr) in enumerate(mm):
            st = i == 0
            sp = i == len(mm) - 1
            if dr:
                rhs3 = X[:, m0 : m0 + mw].rearrange("p (j n) -> p j n", j=2)
                nc.tensor.matmul(pm[0:1, : mw // 2], onesdr, rhs3,
                                 start=st, stop=sp, perf_mode=mybir.MatmulPerfMode.DoubleRow)
            else:
                nc.tensor.matmul(pm[0:1, :mw], ones8[:, 0:1], X[:, m0 : m0 + mw],
                                 start=st, stop=sp)

        # ---- epilogue (big tiles + totals) ----
        nc.vector.tensor_reduce(
            se_big[:, :], sacc[:].rearrange("p (t c) -> p t c", t=NBIG), AX.X, OP.add
        )
        nc.scalar.activation(lse_big[:, :], se_big[:, :], AF.Ln)
        nc.vector.tensor_reduce(red2[:, 0:1], dacc[:, :], AX.X, OP.add)
        nc.vector.tensor_reduce(red2[:, 1:2], lse_big[:, :], AX.X, OP.add)
        nc.tensor.matmul(p2[0:1, :], onesP[:, 0:1], red2[:, :], start=True, stop=True)

        nc.vector.tensor_copy(ot[0:1, 0:1], p2[0:1, 0:1])      # dot total
        # sumlog reduce on ACT (idle at the tail): its dep is the PE sweep's
        # last matmul, and on DVE the scheduler hoists it into the stt queue
        # where it head-of-line blocks for ~6us
        nc.scalar.activation(
            jsum[0:1, :], pm[0:1, :], AF.Copy, accum_out=ot[0:1, 1:2]
        )
        nc.vector.tensor_copy(ot[0:1, 2:3], p2[0:1, 1:2])      # big lse sum (w=1)
        nc.vector.tensor_copy(ot[0:1, 3:4], pw[0:1, :])        # mini w*lse sum
        nc.sync.dma_start(out[0:1, :], ot[0:1, :])

    orig_tables = bacc.get_activation_tables
    bacc.get_activation_tables = _act_tables_ln_exp
    try:
        nc.compile()
    finally:
        bacc.get_activation_tables = orig_tables
    return nc


def _get_prog():
    if "p" not in _PROG_CACHE:
        _PROG_CACHE["p"] = _build()
    return _PROG_CACHE["p"]


def _pack_core(tok_x, tok_s, n_valid):
    """tok_x/tok_s: [SLOTS, V] f32 (zero-padded).

    Slot order: slots 0..255 -> big tokens (tile t partition p = slot 128t+p);
    slots 256..303 -> mini token m: group m//16, octet m%16.
    Returns xr, sr [128, W] fp8 and w16 [16, MG] mask."""
    big = tok_x[: NBIG * P].reshape(NBIG, P, VOCAB)
    bigs = tok_s[: NBIG * P].reshape(NBIG, P, VOCAB)
    mini = tok_x[NBIG * P :].reshape(MG, TPG, SPLIT, C)
    minis = tok_s[NBIG * P :].reshape(MG, TPG, SPLIT, C)

    xr = np.empty((P, W), np.float32)
    sr = np.empty((P, W), np.float32)
    xr[:, :MINIW] = mini.transpose(1, 2, 0, 3).reshape(P, MINIW)
    sr[:, :MINIW] = minis.transpose(1, 2, 0, 3).reshape(P, MINIW)
    for t in range(NBIG):
        xr[:, MINIW + t * VOCAB : MINIW + (t + 1) * VOCAB] = big[t]
        sr[:, MINIW + t * VOCAB : MINIW + (t + 1) * VOCAB] = bigs[t]

    w16 = np.zeros((TPG, MG), np.float32)
    mini_idx = NBIG * P + np.arange(MG * TPG).reshape(MG, TPG).T
    w16[mini_idx < n_valid] = 1.0
    return xr.astype(NP_F8), sr.astype(NP_F8), w16


def _shard(logits, ys, soft_labels, ylens):
    B, T, V = logits.shape
    fl = np.asarray(logits).reshape(B * T, V)
    fs = np.asarray(soft_labels).reshape(B * T, V)
    fy = np.asarray(ys).reshape(B * T)
    yl = np.asarray(ylens).reshape(B)
    valid = (np.arange(T)[None, :] < yl[:, None]).reshape(B * T)
    idx = np.flatnonzero(valid)
    nv = int(idx.size)
    per = math.ceil(nv / NCORES)
    assert per <= SLOTS, f"tokens per core {per} exceed {SLOTS} slots"
    assert per > NBIG * P, "big tiles must be fully valid"

    # exact host-side s_y over valid tokens
    s_y = float(np.sum(fl[idx, fy[idx]], dtype=np.float64))

    p8 = np.zeros((P, TPG), np.float32)
    p8[np.arange(P), np.arange(P) // SPLIT] = 1.0

    in_maps = []
    for c in range(NCORES):
        sel = idx[c * per : (c + 1) * per]
        n = len(sel)
        tx = np.zeros((SLOTS, V), np.float32)
        ts = np.zeros((SLOTS, V), np.float32)
        tx[:n] = fl[sel]
        ts[:n] = fs[sel] * SSCALE
        xr, sr, w16 = _pack_core(tx, ts, n)
        in_maps.append({"xd": xr, "sd": sr, "p8": p8, "wd": w16})
    return in_maps, (B, V, s_y)


def _combine(per_core_outs, B, V, s_y):
    S = np.zeros(4, np.float64)
    for o in per_core_outs:
        S += np.asarray(o, dtype=np.float64).reshape(-1)
    s_dot = S[0] / SSCALE
    s_sumlog = S[1]
    s_wlse = S[2] + S[3]
    c_s = LSM / (V - 1)
    c_y = (1.0 - LSM) - c_s
    t_soft = s_dot - s_wlse
    t_hard = c_y * s_y + c_s * s_sumlog - s_wlse
    loss_soft = -t_soft / B
    loss_hard = -t_hard / B
    loss = SOFT_W * loss_soft + (1.0 - SOFT_W) * loss_hard
    return np.array([loss, loss_soft, loss_hard], dtype=np.float32)


def kernel(logits, ys, soft_labels, ylens):
    global LAST_RESULT
    logits = np.ascontiguousarray(np.asarray(logits), dtype=np.float32)
    soft_labels = np.ascontiguousarray(np.asarray(soft_labels), dtype=np.float32)
    in_maps, (B, V, s_y) = _shard(logits, ys, soft_labels, ylens)
    nc = _get_prog()
    res = run_bass_kernel_spmd(nc, in_maps, list(range(NCORES)))
    LAST_RESULT = res
    return _combine([r["out"] for r in res.results], B, V, s_y)


# ---------------- numpy simulation of the device program ----------------

def _simulate_core(xr, sr, w16):
    x = xr.astype(np.float64)
    s = sr.astype(np.float64)
    dot = (x * s).sum()
    sumlog = x.sum()
    # mini
    em = np.exp(x[:, :MINIW]).astype(ml_dtypes.bfloat16).astype(np.float64)
    sem = em.reshape(P, MG, C).sum(axis=2)
    se16 = sem.reshape(TPG, SPLIT, MG).sum(axis=1)
    wlse_mini = (np.log(se16) * w16).sum()
    # big
    lse_big = 0.0
    for t in range(NBIG):
        eb = np.exp(x[:, MINIW + t * VOCAB : MINIW + (t + 1) * VOCAB])
        lse_big += np.log(eb.sum(axis=1)).sum()
    return dot, sumlog, lse_big, wlse_mini


def simulate(logits, ys, soft_labels, ylens):
    in_maps, (B, V, s_y) = _shard(logits, ys, soft_labels, ylens)
    outs = [np.array(_simulate_core(m["xd"], m["sd"], m["wd"])) for m in in_maps]
    return _combine(outs, B, V, s_y)


if __name__ == "__main__":
    import reference

    ins = reference.setup_inputs()
    exp = np.asarray([float(v) for v in reference.reference(**ins)])
    got = simulate(**{k: np.asarray(v) for k, v in ins.items()})
    rel = np.max(np.abs(got.astype(np.float64) - exp) / np.abs(exp))
    print("expected:", exp)
    print("simulated:", got)
    print(f"sim relative error: {rel:.3e}")
